# revision 34
# baseline (speedup 1.0000x reference)
"""MultiHeadAttention TRN2 Bass kernel.

Full-input contract: kernel(**inputs) takes the unsharded tensors from
setup_inputs() and returns the full [4, 2048, 512] output.

Sharding: 8 cores = 4 batches x 2 query-halves. Each core computes its own
[1024, 512] slice of the output for one batch over all 8 heads, so the
gather is a pure concatenation (no collectives, no all-reduce).

Structure:
  * Inputs are cast to bf16 on the HOST; q/k/v/W reach SBUF transposed via
    13 batched DMA-XBAR transposes straight from DRAM (3D contiguous
    dsts).  No PE transposes, no on-chip fp32->bf16 casts, and half the
    input HBM traffic of the fp32 version.  All transposes ride ONE queue
    (sync): concurrent XBAR transposes issued from two queues corrupt
    each other on HW.  A throwaway 16-row transpose leads the batch to
    absorb the one-time ~10.4us DMA-ring semaphore grant.
  * ACT runs the 128 [128,1024] exps (the ~142us pacer) plus head-7's
    rowsum reciprocal (1/s = exp(-ln s); ln/exp share an act table).
  * One [128,1024] fp32 PSUM ring (ps_s, 4 banks) hosts scores AND the
    projection tiles; AV accumulators (ps_o) take the other 4 banks and
    hand them to the out-projection psum in the tail.
  * Scores run at exp pace; projection quarters are popped at fixed
    slots (2 per slot in heads 0-1); AV lags one full head, which
    satisfies the E-ring / po-ring / V-availability emission-order
    constraints by construction.  Heads 0-6 rowsum reciprocals are exact
    DVE [1,256] chunks; head 7 (tail-critical) uses ACT ln/exp.
  * Tail: pair-3 normalize in 256-col chunks pipelined with the
    out-projection and per-128-row output DMA.
"""
import contextlib

import numpy as np

import bass_rust
import concourse.bass as bass
import concourse.mybir as mybir
import concourse.tile as tile
from concourse.bass_utils import run_bass_kernel_spmd
from concourse.tile import add_dep_helper

F32 = mybir.dt.float32
BF16 = mybir.dt.bfloat16

B, S, D_MODEL = 4, 2048, 512
NUM_HEADS = 8
HEAD_DIM = 64
SQ = S // 2  # queries per core
N_CORES = 8
SCALE = 1.0 / 8.0  # 1/sqrt(HEAD_DIM)

_split_ctr = [0]


def split_waits(nc, max_waits: int = 1):
    """walrus codegen rejects instructions carrying >1 sync wait; move the
    extras onto standalone EventSemaphore instructions on the same engine."""
    for f in nc.m.functions:
        for blk in f.blocks:
            new_insts = []
            changed = False
            for inst in blk.instructions:
                si = inst.sync_info
                if si is not None and si.on_wait and len(si.on_wait) > max_waits:
                    waits = list(si.on_wait)
                    extra, keep = waits[:-max_waits], waits[-max_waits:]
                    for w in extra:
                        _split_ctr[0] += 1
                        ev = mybir.InstEventSemaphore(
                            name=f"I-wsplit-{_split_ctr[0]}", ins=[], outs=[]
                        )
                        ev.engine = inst.engine
                        ev.sync_info = bass_rust.SyncInfo(on_wait=[w], on_update=[])
                        new_insts.append(ev)
                    inst.sync_info = bass_rust.SyncInfo(
                        on_wait=keep, on_update=list(si.on_update)
                    )
                    changed = True
                new_insts.append(inst)
            if changed:
                blk.instructions = new_insts


def build_mha():
    nc = bass.Bass("TRN2", target_bir_lowering=False, debug=False, num_devices=1)

    qd = nc.declare_dram_parameter("q", [SQ, D_MODEL], BF16, isOutput=False).ap()
    kd = nc.declare_dram_parameter("k", [S, D_MODEL], BF16, isOutput=False).ap()
    vd = nc.declare_dram_parameter("v", [S, D_MODEL], BF16, isOutput=False).ap()
    wts = {
        n: nc.declare_dram_parameter(n, [D_MODEL, D_MODEL], BF16, isOutput=False).ap()
        for n in ("wq", "wk", "wv", "wo")
    }
    bias = {
        n: nc.declare_dram_parameter(n, [D_MODEL], F32, isOutput=False).ap()
        for n in ("bq", "bk", "bv", "bo")
    }
    outd = nc.declare_dram_parameter("out", [SQ, D_MODEL], F32, isOutput=True).ap()

    H2 = NUM_HEADS // 2  # head pairs = dout tiles of 128
    KTILES = S // 128  # 16
    EH_SLOTS = 24  # ring depth for E tiles (1.5 heads of slack)

    # filler pacing per score slot (one slot = 2 exps = ~2.23us on ACT):
    # the scores cost 4 matmuls; ~6.2 more matmuls and ~2us of DVE fit per
    # slot.  Credits bank so bursts catch up after starved slots.
    PE_RATE, PE_CAP = 6.4, 9.0
    DVE_RATE, DVE_CAP = 2.0, 4.0

    with tile.TileContext(nc) as tc, contextlib.ExitStack() as top:
        consts = top.enter_context(tc.tile_pool(name="consts", bufs=1))
        wt_pool = top.enter_context(tc.tile_pool(name="wt", bufs=1))
        xt_pool = top.enter_context(tc.tile_pool(name="xt", bufs=1))
        proj_out = top.enter_context(tc.tile_pool(name="proj_out", bufs=1))
        epilog = top.enter_context(tc.tile_pool(name="epilog", bufs=1))
        rsrf_pool = top.enter_context(tc.tile_pool(name="rsrf", bufs=1))
        ehpool = top.enter_context(tc.tile_pool(name="ehpool", bufs=EH_SLOTS))
        # one [128,1024] fp32 psum ring for scores + projections (banks 0-3)
        ps_s = top.enter_context(tc.tile_pool(name="ps_s", bufs=2, space="PSUM"))
        # AV accumulators (banks 4-7); closed manually in the tail so the
        # out-projection psum can take over its banks
        ps_o_stack = contextlib.ExitStack()
        ps_o = ps_o_stack.enter_context(tc.tile_pool(name="ps_o", bufs=2, space="PSUM"))

        # ---- constants (gpsimd queue so sync/scalar stay free for XBARs)
        bqt = consts.tile([128, 4], F32)
        bkt = consts.tile([128, 4], F32)
        for t_, name in ((bqt, "bq"), (bkt, "bk")):
            nc.gpsimd.dma_start(
                out=t_, in_=bias[name].rearrange("(c p) -> p c", p=128)
            )
        bvb = consts.tile([128, D_MODEL], F32)
        bob = consts.tile([128, D_MODEL], F32)
        for t_, name in ((bvb, "bv"), (bob, "bo")):
            src = bias[name]
            nc.gpsimd.dma_start(
                out=t_,
                in_=bass.AP(tensor=src.tensor, offset=src.offset, ap=[[0, 128], [1, D_MODEL]]),
            )
        # upper/lower-half selection rows for the rowsum broadcast matmuls
        e_up = consts.tile([1, 128], BF16)
        e_dn = consts.tile([1, 128], BF16)
        nc.vector.memset(e_up, 0.0)
        nc.vector.memset(e_up[:, 0:HEAD_DIM], 1.0)
        nc.vector.memset(e_dn, 0.0)
        nc.vector.memset(e_dn[:, HEAD_DIM:128], 1.0)

        # ---- long-lived tiles
        # W^T: WTB[name][p, dc, dout] = W[dout, dc*128+p]
        WTB = {
            n: wt_pool.tile([128, 4, D_MODEL], BF16, name=f"wt_{n}", tag=f"wt_{n}")
            for n in ("wq", "wk", "wv", "wo")
        }
        # x^T chunks: [p, dc, rows]; xT[din=dc*128+p, row].  k/q come in
        # 512-row tiles (one DMA-transpose each, fully contiguous dst) so
        # the first projections can start as early as possible.
        KXT = [
            xt_pool.tile([128, 4, 1024], BF16, name=f"kxt_{i}", tag=f"kxt_{i}")
            for i in range(2)
        ]
        QXT = xt_pool.tile([128, 4, SQ], BF16, name="qxt", tag="qxt")
        VXT = [
            xt_pool.tile([128, 4, 1024], BF16, name=f"vxt_{i}", tag=f"vxt_{i}")
            for i in range(2)
        ]

        def kxt(dc, c):  # 512-row chunk c of k^T, din chunk dc -> [128, 512]
            return KXT[c // 2][:, dc, (c % 2) * 512 : (c % 2) * 512 + 512]

        def vxt(dc, sc):  # 128-row chunk sc of v^T -> [128, 128]
            return VXT[sc // 8][:, dc, (sc % 8) * 128 : (sc % 8) * 128 + 128]

        QTz = [
            proj_out.tile([128, SQ], BF16, name=f"qtz_{h}", tag=f"qtz_{h}")
            for h in range(NUM_HEADS)
        ]
        KT = [proj_out.tile([128, S], BF16, name=f"kt_{t}", tag=f"kt_{t}") for t in range(H2)]
        V = [
            proj_out.tile([128, NUM_HEADS, HEAD_DIM + 1], BF16, name=f"v_{sc}", tag=f"v_{sc}")
            for sc in range(KTILES)
        ]
        OU = [epilog.tile([128, SQ], F32, name=f"ou_{t}", tag=f"ou_{t}") for t in range(H2)]
        OMT = [epilog.tile([128, SQ], BF16, name=f"omt_{t}", tag=f"omt_{t}") for t in range(H2)]
        RSR = [
            epilog.tile([1, SQ], BF16, name=f"rsr_{h}", tag=f"rsr_{h}")
            for h in range(NUM_HEADS)
        ]

        # QTz zero halves: only the half head h does NOT write needs zeros
        for h in range(NUM_HEADS):
            z0, z1 = (HEAD_DIM, 128) if h % 2 == 0 else (0, HEAD_DIM)
            nc.vector.memset(QTz[h][z0:z1, :], 0.0)
        # V rowsum ones column, written once (bias add only touches 0:64)
        for sc in range(KTILES):
            nc.gpsimd.memset(V[sc][:, :, HEAD_DIM : HEAD_DIM + 1], 1.0)

        # ---- batched DMA-XBAR transposes straight from DRAM, ALL on the
        # sync queue: concurrent XBAR transposes from two queues corrupt
        # each other on HW (the tile framework only guards same-queue and
        # DT-vs-SBUF-DMA hazards).  Ordered so the head-0 dependency chain
        # (wk, k chunk 0, wq, q) lands first.  Each instr has a fully
        # contiguous 3D dst: out[p, dc, m] = in[m, dc*128+p].
        # A throwaway 16-row transpose goes first: the first DT pays a
        # ~10.4us one-time DMA-ring semaphore grant before its successor
        # may issue; the dummy absorbs it.
        dt_dummy = consts.tile([128, 16], BF16, name="dt_dummy", tag="dt_dummy")
        nc.sync.dma_start(out=dt_dummy, in_=kd[0:16, 0:128], transpose=True)
        nc.sync.dma_start(out=WTB["wk"], in_=wts["wk"], transpose=True)
        nc.sync.dma_start(out=KXT[0], in_=kd[0:1024, :], transpose=True)
        nc.sync.dma_start(out=WTB["wq"], in_=wts["wq"], transpose=True)
        nc.sync.dma_start(out=QXT, in_=qd, transpose=True)
        nc.sync.dma_start(out=KXT[1], in_=kd[1024:2048, :], transpose=True)
        nc.sync.dma_start(out=WTB["wv"], in_=wts["wv"], transpose=True)
        nc.sync.dma_start(out=VXT[0], in_=vd[0:1024, :], transpose=True)
        nc.sync.dma_start(out=VXT[1], in_=vd[1024:2048, :], transpose=True)
        nc.sync.dma_start(out=WTB["wo"], in_=wts["wo"], transpose=True)

        pe_chain = [None]

        def chain(bi):
            if pe_chain[0] is not None:
                add_dep_helper(bi.ins, pe_chain[0].ins, reason="pe-order")
            pe_chain[0] = bi

        # E-tile ring: slot per (head, kc)
        eh_slots = {}

        def eh_slot(h, kc):
            key = (h, kc)
            if key not in eh_slots:
                eh_slots[key] = ehpool.tile(
                    [128, SQ], BF16, name=f"eh_{h}_{kc}", tag="eh"
                )
            return eh_slots[key]

        # AV psum accumulators, allocated lazily in pop order
        po_tiles = {}

        def get_po(h):
            if h not in po_tiles:
                po_tiles[h] = ps_o.tile([HEAD_DIM + 1, SQ], F32, name=f"po_{h}", tag="po")
            return po_tiles[h]

        # ---------------- building blocks (4-matmul quarters) ----------------
        def k_q(c, t):
            """KT[t] for 512-row chunk c (4 matmuls, 1 ring tile)."""
            pj = ps_s.tile([128, 512], F32, tag="pscore")
            for dc in range(4):
                chain(
                    nc.tensor.matmul(
                        pj,
                        WTB["wk"][:, dc, t * 128 : (t + 1) * 128],
                        kxt(dc, c),
                        start=(dc == 0),
                        stop=(dc == 3),
                    )
                )
            nc.vector.tensor_scalar_add(
                KT[t][:, c * 512 : (c + 1) * 512], pj, bkt[:, t : t + 1]
            )

        def q_q(c, t):
            """QTz[2t], QTz[2t+1] slices for q chunk c (4 matmuls)."""
            sl = slice(c * 512, (c + 1) * 512)
            pj = ps_s.tile([128, 512], F32, tag="pscore")
            for dc in range(4):
                chain(
                    nc.tensor.matmul(
                        pj,
                        WTB["wq"][:, dc, t * 128 : (t + 1) * 128],
                        QXT[:, dc, c * 512 : (c + 1) * 512],
                        start=(dc == 0),
                        stop=(dc == 3),
                    )
                )
            nc.vector.tensor_scalar_add(
                QTz[2 * t][0:HEAD_DIM, sl],
                pj[0:HEAD_DIM, :],
                bqt[0:HEAD_DIM, t : t + 1],
            )
            nc.vector.tensor_scalar_add(
                QTz[2 * t + 1][HEAD_DIM:128, sl],
                pj[HEAD_DIM:128, :],
                bqt[HEAD_DIM:128, t : t + 1],
            )

        def v_q(sc):
            """V[sc] (4 matmuls, 1 ring tile)."""
            pj = ps_s.tile([128, 512], F32, tag="pscore")
            for dc in range(4):
                chain(
                    nc.tensor.matmul(
                        pj,
                        vxt(dc, sc),
                        WTB["wv"][:, dc, :],
                        start=(dc == 0),
                        stop=(dc == 3),
                    )
                )
            nc.vector.tensor_add(
                V[sc][:, :, 0:HEAD_DIM],
                pj.rearrange("p (h d) -> p h d", h=NUM_HEADS),
                bvb.rearrange("p (h d) -> p h d", h=NUM_HEADS),
            )

        def emit_scores_kb(h, kb):
            """4 score matmuls + 2 exps for (head h, k-batch kb)."""
            t = h // 2
            pss = []
            for j in range(2):
                kc = 2 * kb + j
                pscore = ps_s.tile([128, SQ], F32, tag="pscore")
                for qc in range(SQ // 512):
                    sl = slice(qc * 512, (qc + 1) * 512)
                    chain(
                        nc.tensor.matmul(
                            pscore[:, sl],
                            KT[t][:, kc * 128 : (kc + 1) * 128],
                            QTz[h][:, sl],
                            start=True,
                            stop=True,
                        )
                    )
                pss.append((kc, pscore))
            for kc, pscore in pss:
                nc.scalar.activation(
                    eh_slot(h, kc),
                    pscore,
                    mybir.ActivationFunctionType.Exp,
                    scale=SCALE,
                )

        def emit_av(h, kb):
            po = get_po(h)
            for j in range(2):
                kc = 2 * kb + j
                peh = eh_slot(h, kc)
                for qc in range(SQ // 512):
                    sl = slice(qc * 512, (qc + 1) * 512)
                    chain(
                        nc.tensor.matmul(
                            po[:, sl],
                            V[kc][:, h, :],
                            peh[:, sl],
                            start=(kc == 0),
                            stop=(kc == KTILES - 1),
                        )
                    )

        def finalize(h):
            t, half = h // 2, h % 2
            po = get_po(h)
            with nc.allow_low_precision("softmax denominators in bf16"):
                for qq in range(4):
                    sl = slice(qq * 256, (qq + 1) * 256)
                    nc.vector.reciprocal(
                        RSR[h][:, sl], po[HEAD_DIM : HEAD_DIM + 1, sl]
                    )
                    if qq == 0:
                        nc.vector.tensor_copy(
                            OU[t][half * HEAD_DIM : (half + 1) * HEAD_DIM, :],
                            po[0:HEAD_DIM, :],
                        )
            for kc in range(KTILES):
                eh_slots.pop((h, kc), None)

        def pair_normalize(t):
            pr = ps_s.tile([128, SQ], F32, tag="pscore")
            for qc in range(2):
                sl = slice(qc * 512, (qc + 1) * 512)
                chain(nc.tensor.matmul(pr[:, sl], e_up, RSR[2 * t][:, sl], start=True, stop=False))
                chain(nc.tensor.matmul(pr[:, sl], e_dn, RSR[2 * t + 1][:, sl], start=False, stop=True))
            nc.vector.tensor_mul(OMT[t], OU[t], pr)

        # ---------------- explicit schedule ----------------
        # Projection quarters are popped at fixed slots (2 per score slot
        # in heads 0-1, 1 per slot early in head 2); AV lags one full head
        # (v1-proven: satisfies the E-ring, po-ring and V-availability
        # emission-order constraints by construction).
        S0 = [
            (k_q, 1, 0), (k_q, 2, 0), (k_q, 3, 0),
            (k_q, 0, 1), (q_q, 0, 1), (q_q, 1, 1),
            (v_q, 0), (v_q, 1), (v_q, 2), (v_q, 3),
            (v_q, 4), (v_q, 5), (k_q, 1, 1), (k_q, 2, 1),
            (v_q, 6), (v_q, 7),
        ]
        S1 = [
            (k_q, 3, 1), (v_q, 8), (v_q, 9), (v_q, 10), (v_q, 11),
            (v_q, 12), (v_q, 13), (v_q, 14), (v_q, 15),
            (k_q, 0, 2), (k_q, 1, 2), (q_q, 0, 2), (k_q, 2, 2),
            (k_q, 3, 2), (q_q, 1, 2), (k_q, 0, 3),
        ]
        S2 = [
            (k_q, 1, 3), (q_q, 0, 3), (k_q, 2, 3), (k_q, 3, 3), (q_q, 1, 3),
        ]

        def pop(queue, n):
            for _ in range(n):
                if queue:
                    u = queue.pop(0)
                    u[0](*u[1:])

        # eager prefix: KT[0] chunk 0 + QTz[0,1] chunks
        k_q(0, 0)
        q_q(0, 0)
        q_q(1, 0)

        # ---- head 0: scores + 2 projection quarters per slot
        for kb in range(KTILES // 2):
            emit_scores_kb(0, kb)
            pop(S0, 2)

        # ---- head 1: scores + AV(0) + 2 quarters per slot
        for kb in range(KTILES // 2):
            emit_scores_kb(1, kb)
            emit_av(0, kb)
            pop(S1, 2)
        finalize(0)

        # ---- heads 2..7 (v1-proven lag/special-casing)
        for h in range(2, NUM_HEADS):
            prev_h = h - 1
            for kb in range(KTILES // 2):
                emit_scores_kb(h, kb)
                if h == NUM_HEADS - 1:
                    # head 6's AV remainder, then head 7's AV at 2-kb lag
                    if kb < 2:
                        emit_av(prev_h, kb + 6)
                    if kb == 2:
                        finalize(prev_h)
                    if kb >= 2:
                        emit_av(h, kb - 2)
                else:
                    emit_av(prev_h, kb)
                    if h == NUM_HEADS - 2 and kb >= 2:
                        emit_av(h, kb - 2)
                if h == 2:
                    pop(S2, 1)
                if kb == 5 and h in (3, 5):
                    pair_normalize((h - 3) // 2)
            if h != NUM_HEADS - 1:
                finalize(prev_h)
        for kb in range(KTILES // 2 - 2, KTILES // 2):
            emit_av(NUM_HEADS - 1, kb)

        # ============= tail: head-7 finalize + out projection =============
        # head 7 reciprocal on ACT: 1/s = exp(-ln s); ln/exp share the
        # loaded act table and ACT is idle after the last score exp
        po7 = get_po(NUM_HEADS - 1)
        nc.vector.tensor_copy(OU[H2 - 1][HEAD_DIM:128, :], po7[0:HEAD_DIM, :])
        # pair-2 normalize here instead of inside head 7's score chain: its
        # pr tile borrowed the score-psum ring there and its DVE multiply
        # stalled the last score slots; here it overlaps the ACT ln/exp
        pair_normalize(2)
        lnr = rsrf_pool.tile([1, SQ], F32, tag="rsrf")
        with nc.allow_low_precision("softmax denominators in bf16"):
            nc.scalar.activation(
                lnr,
                po7[HEAD_DIM : HEAD_DIM + 1, :],
                mybir.ActivationFunctionType.Ln,
            )
            nc.scalar.activation(
                RSR[NUM_HEADS - 1],
                lnr,
                mybir.ActivationFunctionType.Exp,
                scale=-1.0,
            )
        ps_o_stack.close()  # free AV psum banks for the out-projection
        with (
            tc.tile_pool(name="outsb", bufs=2) as outsb,
            tc.tile_pool(name="ps_f", bufs=2, space="PSUM") as ps_f,
        ):
            t3 = H2 - 1
            pr = ps_s.tile([128, SQ], F32, tag="pscore")
            for qq in range(4):
                sl = slice(qq * 256, (qq + 1) * 256)
                chain(nc.tensor.matmul(pr[:, sl], e_up, RSR[2 * t3][:, sl], start=True, stop=False))
                chain(nc.tensor.matmul(pr[:, sl], e_dn, RSR[2 * t3 + 1][:, sl], start=False, stop=True))
                nc.vector.tensor_mul(OMT[t3][:, sl], OU[t3][:, sl], pr[:, sl])
                for sq in range(qq * 2, qq * 2 + 2):
                    pf = ps_f.tile([128, D_MODEL], F32, tag="pf")
                    for t in range(H2):
                        chain(
                            nc.tensor.matmul(
                                pf,
                                OMT[t][:, sq * 128 : (sq + 1) * 128],
                                WTB["wo"][:, t, :],
                                start=(t == 0),
                                stop=(t == H2 - 1),
                            )
                        )
                    ot = outsb.tile([128, D_MODEL], F32, tag="ot")
                    nc.vector.tensor_add(ot, pf, bob)
                    nc.sync.dma_start(out=outd[sq * 128 : (sq + 1) * 128, :], in_=ot)

    split_waits(nc)
    return nc


_cached_nc = None


def _get_nc():
    global _cached_nc
    if _cached_nc is None:
        _cached_nc = build_mha()
    return _cached_nc


def _make_in_maps(q, k, v, Wq, bq, Wk, bk, Wv, bv, Wo, bo):
    import ml_dtypes

    bf = ml_dtypes.bfloat16
    q = np.asarray(q, dtype=np.float32).astype(bf)
    k = np.asarray(k, dtype=np.float32).astype(bf)
    v = np.asarray(v, dtype=np.float32).astype(bf)
    weights = {
        "wq": np.ascontiguousarray(np.asarray(Wq, np.float32).astype(bf)),
        "wk": np.ascontiguousarray(np.asarray(Wk, np.float32).astype(bf)),
        "wv": np.ascontiguousarray(np.asarray(Wv, np.float32).astype(bf)),
        "wo": np.ascontiguousarray(np.asarray(Wo, np.float32).astype(bf)),
        "bq": np.ascontiguousarray(np.asarray(bq, np.float32)),
        "bk": np.ascontiguousarray(np.asarray(bk, np.float32)),
        "bv": np.ascontiguousarray(np.asarray(bv, np.float32)),
        "bo": np.ascontiguousarray(np.asarray(bo, np.float32)),
    }
    in_maps = []
    for core in range(N_CORES):
        b, qh = core // 2, core % 2
        in_maps.append(
            {
                "q": np.ascontiguousarray(q[b, qh * SQ : (qh + 1) * SQ, :]),
                "k": np.ascontiguousarray(k[b]),
                "v": np.ascontiguousarray(v[b]),
                **weights,
            }
        )
    return in_maps


def kernel(q, k, v, mask, Wq, bq, Wk, bk, Wv, bv, Wo, bo, **_unused):
    in_maps = _make_in_maps(q, k, v, Wq, bq, Wk, bk, Wv, bv, Wo, bo)
    nc = _get_nc()
    res = run_bass_kernel_spmd(nc, in_maps, list(range(N_CORES)))
    out = np.empty((B, S, D_MODEL), dtype=np.float32)
    for core in range(N_CORES):
        b, qh = core // 2, core % 2
        out[b, qh * SQ : (qh + 1) * SQ, :] = res.results[core]["out"]
    return out
